# revision 1
# baseline (speedup 1.0000x reference)
"""GarNet layer kernel for Trainium2 (8 NeuronCores, data-parallel over batch).

Math (per example b):
    w    = exp(-d_av^2)                      [V=128, S=16]
    hi   = w^T @ fi_v / V                    [S, N=64]
    out  = mean_V(w)[:, None] * hi           [S, N] -> flattened [S*N]

Implementation notes:
  - Batch B=4096 is sharded 512/core across 8 cores (pure data parallel).
  - Per example, one fp32 matmul: lhsT = w [V=128, S=16], rhs = fi
    augmented with a constant column of 1/V^2, so PSUM column N holds
    sum_V(w)/V^2 and the final output is just psum[:, :N] * psum[:, N]
    per partition (exactly the reference quantity).
  - Four examples share one PSUM bank at partition offsets {0,32,64,96}
    via tile_position col-tiling, so the epilogue runs on 128-partition
    tiles and the four matmuls overlap in distinct PE column groups.
"""

import numpy as np
from contextlib import ExitStack

import concourse.bass as bass
import concourse.tile as tile
from concourse import mybir
from concourse.bass_utils import run_bass_kernel_spmd

B, V, S, N = 4096, 128, 16, 64
NCORES = 8
BPC = B // NCORES            # examples per core
ONES_VAL = 1.0 / (V * V)     # exact power of two; folds /V^2 into the matmul


def split_multi_waits(nc):
    """The walrus build in this container rejects >1 embedded sem-wait per
    instruction ("Too many sync wait commands" in setupSyncWait). Hoist every
    multi-wait list onto single-wait EventSemaphore instructions immediately
    before the owner on the same engine — identical semantics, since engine
    streams are in order."""
    fn = nc.m.functions[0]
    for block in fn.blocks:
        insts = list(block.instructions)
        changed = False
        new = []
        for inst in insts:
            si = inst.sync_info
            waits = list(si.on_wait) if (si and si.on_wait) else []
            if len(waits) > 1:
                changed = True
                for w in waits:
                    ev = mybir.InstEventSemaphore(
                        name=nc.get_next_instruction_name(), ins=[], outs=[]
                    )
                    ev.engine = inst.engine
                    ev.sync_info = mybir.SyncInfo(on_wait=[w], on_update=[])
                    new.append(ev)
                ups = list(si.on_update) if si.on_update else []
                inst.sync_info = mybir.SyncInfo(on_wait=[], on_update=ups)
            new.append(inst)
        if changed:
            block.instructions = new


def build(bpc=BPC, e_chunk=32, name="garnet", split_waits=True):
    """Build the per-core Bass module for a shard of `bpc` examples.

    split_waits: apply the walrus multi-wait workaround (needed for HW
    compile; leave False for CoreSim, whose race detector doesn't know
    about post-hoc instructions).
    """
    assert bpc % e_chunk == 0 and e_chunk % 8 == 0
    nchunk = bpc // e_chunk
    G = e_chunk // 8   # psum groups (8 examples each) per chunk
    Q = e_chunk // 2   # w pairs per chunk

    nc = bass.Bass(name=name)
    fi = nc.dram_tensor("fi_v", (bpc, V, N), mybir.dt.float32, kind="ExternalInput")
    dav = nc.dram_tensor("d_av", (bpc, V, S), mybir.dt.float32, kind="ExternalInput")
    out = nc.dram_tensor("out", (bpc, S * N), mybir.dt.float32, kind="ExternalOutput")

    f32 = mybir.dt.float32
    with tile.TileContext(nc) as tc, ExitStack() as ctx:
        fipool = ctx.enter_context(tc.tile_pool(name="fipool", bufs=2))
        dpool = ctx.enter_context(tc.tile_pool(name="dpool", bufs=2))
        opool = ctx.enter_context(tc.tile_pool(name="opool", bufs=2))
        colpool = ctx.enter_context(tc.tile_pool(name="colpool", bufs=4))
        psum = ctx.enter_context(tc.tile_pool(name="psum", bufs=8, space="PSUM"))

        for c in range(nchunk):
            b0 = c * e_chunk
            # fi chunk -> [V, e, N+1]; col N = 1/V^2 for the wbar column
            fi_t = fipool.tile([128, e_chunk, N + 1], f32)
            nc.vector.memset(fi_t[:, :, N : N + 1], ONES_VAL)
            nc.sync.dma_start(
                out=fi_t[:, :, 0:N],
                in_=fi[b0 : b0 + e_chunk].rearrange("e v n -> v e n"),
            )
            # d chunk -> [V, pair, slot, S] with slot layout [w_2q, ZERO, w_2q+1];
            # then w = exp(-d^2) on the two w slots only (zeros stay zero).
            # Each matmul then takes a 32-wide lhsT: pair-even = (w_a, Z),
            # pair-odd = (Z, w_b). With PSUM accumulate (start only on the
            # bank's first matmul), the zero half writes/accumulates zeros, so
            # 8 examples pack one bank at rows 16*jj with no junk rows.
            d_t = dpool.tile([128, Q, 3, S], f32)
            nc.vector.memset(d_t[:, :, 1, :], 0.0)
            dsrc = dav[b0 : b0 + e_chunk].rearrange("(q t) v s -> t v q s", t=2)
            for t in range(2):
                nc.sync.dma_start(out=d_t[:, :, 2 * t, :], in_=dsrc[t])
                nc.vector.tensor_mul(
                    d_t[:, :, 2 * t, :], d_t[:, :, 2 * t, :], d_t[:, :, 2 * t, :]
                )
                nc.scalar.activation(
                    d_t[:, :, 2 * t, :],
                    d_t[:, :, 2 * t, :],
                    mybir.ActivationFunctionType.Exp,
                    scale=-1.0,
                )

            o_t = opool.tile([128, G, N], f32)
            for g in range(G):
                ps = psum.tile([128, N + 1], f32)
                for jj in range(8):
                    e = g * 8 + jj          # example within chunk
                    q, t = e // 2, e % 2    # pair index, parity
                    nc.tensor.matmul(
                        out=ps[32 * (jj // 2) : 32 * (jj // 2) + 32, :],
                        lhsT=d_t[:, q, t : t + 2, :],
                        rhs=fi_t[:, e, :],
                        start=(t == 0),
                        stop=(t == 1),
                        tile_position=(0, 32 * (jj // 2)),
                    )
                col = colpool.tile([128, 1], f32)
                nc.scalar.copy(col, ps[:, N : N + 1])
                nc.vector.tensor_scalar_mul(o_t[:, g, :], ps[:, 0:N], col)

            # partition p = 16*jj + s maps linearly to DRAM offset p*256B of
            # example b0+8g+jj -> one full-128-partition DMA per chunk.
            dst = out[b0 : b0 + e_chunk].rearrange(
                "(g jj) (s n) -> (jj s) g n", jj=8, s=S
            )
            nc.sync.dma_start(out=dst, in_=o_t)

    if split_waits:
        split_multi_waits(nc)
    return nc


_NC_CACHE = {}


def _get_nc():
    if "nc" not in _NC_CACHE:
        _NC_CACHE["nc"] = build()
    return _NC_CACHE["nc"]


def kernel(fi_v: np.ndarray, d_av: np.ndarray) -> np.ndarray:
    fi_v = np.ascontiguousarray(np.asarray(fi_v, dtype=np.float32))
    d_av = np.ascontiguousarray(np.asarray(d_av, dtype=np.float32))
    assert fi_v.shape == (B, V, N) and d_av.shape == (B, V, S)
    nc = _get_nc()
    in_maps = [
        {
            "fi_v": fi_v[c * BPC : (c + 1) * BPC],
            "d_av": d_av[c * BPC : (c + 1) * BPC],
        }
        for c in range(NCORES)
    ]
    res = run_bass_kernel_spmd(nc, in_maps, core_ids=list(range(NCORES)))
    return np.concatenate([res.results[c]["out"] for c in range(NCORES)], axis=0)



# revision 5
# speedup vs baseline: 2.0215x; 2.0215x over previous
"""GarNet layer kernel for Trainium2 (8 NeuronCores, data-parallel over batch).

Math (per example b):
    w    = exp(-d_av^2)                      [V=128, S=16]
    hi   = w^T @ fi_v / V                    [S, N=64]
    out  = mean_V(w)[:, None] * hi           [S, N] -> flattened [S*N]

Implementation notes:
  - Batch B=4096 is sharded 512/core across 8 cores (pure data parallel).
  - Inputs are pre-transposed on the HOST to v-major ([V, bpc, N] / [V, bpc, S])
    so every device DMA moves >=2KB contiguous runs per partition; in the TRN2
    cost model descriptors below 512B pay a 2x latency penalty, which dominated
    the previous version (fi loads were 256B runs). The device output tensor is
    likewise stored in the kernel's natural packed layout and unscrambled on
    the host.
  - Per example, one fp32 matmul pair: lhsT is the zero-slotted w pair
    [w_even, ZERO, w_odd], accumulating two 32-col matmuls per 32-row PSUM
    window, so 8 examples pack one PSUM bank at rows 16*jj.
  - wbar (mean over V of w) comes from two extra 1-column matmuls per group
    against a constant 1/V^2 ones vector, using the same zero-slotted lhsT
    windows so the even/odd sums land interleaved on the right partitions.
  - Loads issue on the SP queue, stores on the Pool/SWDGE queue so a store's
    semaphore wait never blocks the next chunk's load issue.
"""

import numpy as np
from contextlib import ExitStack

import concourse.bass as bass
import concourse.tile as tile
from concourse import mybir
from concourse.bass_utils import run_bass_kernel_spmd

B, V, S, N = 4096, 128, 16, 64
NCORES = 8
BPC = B // NCORES            # examples per core
E_CHUNK = 32                 # examples per chunk
ONES_VAL = 1.0 / (V * V)     # exact power of two; folds /V^2 into the matmul


def split_multi_waits(nc):
    """The walrus build in this container rejects >1 embedded sem-wait per
    instruction ("Too many sync wait commands" in setupSyncWait). Hoist every
    multi-wait list onto single-wait EventSemaphore instructions immediately
    before the owner on the same engine — identical semantics, since engine
    streams are in order."""
    fn = nc.m.functions[0]
    for block in fn.blocks:
        insts = list(block.instructions)
        changed = False
        new = []
        for inst in insts:
            si = inst.sync_info
            waits = list(si.on_wait) if (si and si.on_wait) else []
            if len(waits) > 1:
                changed = True
                for w in waits:
                    ev = mybir.InstEventSemaphore(
                        name=nc.get_next_instruction_name(), ins=[], outs=[]
                    )
                    ev.engine = inst.engine
                    ev.sync_info = mybir.SyncInfo(on_wait=[w], on_update=[])
                    new.append(ev)
                ups = list(si.on_update) if si.on_update else []
                inst.sync_info = mybir.SyncInfo(on_wait=[], on_update=ups)
            new.append(inst)
        if changed:
            block.instructions = new


def build(bpc=BPC, e_chunk=E_CHUNK, name="garnet", split_waits=True):
    """Build the per-core Bass module for a shard of `bpc` examples.

    Device I/O layouts (host does the transposes):
      fi_t : [V, bpc, N]  = fi_v shard transposed to v-major
      d_t  : [V, bpc, S]  = d_av shard transposed to v-major
      out  : [128, nchunk, e_chunk//8, N] packed as partition p = 16*jj + s,
             example e = chunk*e_chunk + g*8 + jj
    """
    assert bpc % e_chunk == 0 and e_chunk % 8 == 0
    nchunk = bpc // e_chunk
    Q = e_chunk // 2   # w pairs per chunk
    G = e_chunk // 8   # psum groups (8 examples each) per chunk

    nc = bass.Bass(name=name)
    fiT = nc.dram_tensor("fi_t", (V, bpc, N), mybir.dt.float32, kind="ExternalInput")
    dT = nc.dram_tensor("d_t", (V, bpc, S), mybir.dt.float32, kind="ExternalInput")
    out = nc.dram_tensor(
        "out", (128, nchunk, G, N), mybir.dt.float32, kind="ExternalOutput"
    )

    f32 = mybir.dt.float32
    with tile.TileContext(nc) as tc, ExitStack() as ctx:
        fipool = ctx.enter_context(tc.tile_pool(name="fipool", bufs=3))
        dpool = ctx.enter_context(tc.tile_pool(name="dpool", bufs=2))
        wpool = ctx.enter_context(tc.tile_pool(name="wpool", bufs=2))
        opool = ctx.enter_context(tc.tile_pool(name="opool", bufs=2))
        colpool = ctx.enter_context(tc.tile_pool(name="colpool", bufs=8))
        cpool = ctx.enter_context(tc.tile_pool(name="cpool", bufs=1))
        psum = ctx.enter_context(tc.tile_pool(name="psum", bufs=8, space="PSUM"))

        ones = cpool.tile([128, 1], f32)
        nc.vector.memset(ones, ONES_VAL)

        for c in range(nchunk):
            b0 = c * e_chunk
            # fi chunk: [128, e, 64] fully contiguous per partition -> 8KB runs
            fi_t = fipool.tile([128, e_chunk, N], f32)
            nc.sync.dma_start(out=fi_t, in_=fiT[:, b0 : b0 + e_chunk, :])
            # d chunk: [128, e, 16] contiguous -> 2KB runs
            d_t = dpool.tile([128, e_chunk, S], f32)
            nc.sync.dma_start(out=d_t, in_=dT[:, b0 : b0 + e_chunk, :])

            # w = exp(-d^2) into the zero-slotted pair layout
            # [128, Q, 3, S] with slots [w_even, ZERO, w_odd].
            w_t = wpool.tile([128, Q, 3, S], f32)
            nc.vector.memset(w_t[:, :, 1, :], 0.0)
            nc.vector.tensor_mul(d_t, d_t, d_t)
            nc.scalar.activation(
                w_t[:, :, 0::2, :],
                d_t.rearrange("p (q t) s -> p q t s", t=2),
                mybir.ActivationFunctionType.Exp,
                scale=-1.0,
            )

            o_t = opool.tile([128, G, N], f32)
            for g in range(G):
                ps = psum.tile([128, N + 1], f32)
                pw = ps[:, N : N + 1]
                for jj in range(8):
                    e = g * 8 + jj          # example within chunk
                    q, t = e // 2, e % 2    # pair index, parity
                    nc.tensor.matmul(
                        out=ps[32 * (jj // 2) : 32 * (jj // 2) + 32, 0:N],
                        lhsT=w_t[:, q, t : t + 2, :],
                        rhs=fi_t[:, e, :],
                        start=(t == 0),
                        stop=(t == 1),
                        tile_position=(0, 32 * (jj // 2)),
                    )
                # wbar column: two slot-masked 1-col matmuls accumulate
                # sum_V(w)/V^2 for even then odd examples on interleaved rows.
                nc.tensor.matmul(
                    out=pw,
                    lhsT=w_t[:, 4 * g : 4 * g + 4, 0:2, :],
                    rhs=ones,
                    start=True,
                    stop=False,
                )
                nc.tensor.matmul(
                    out=pw,
                    lhsT=w_t[:, 4 * g : 4 * g + 4, 1:3, :],
                    rhs=ones,
                    start=False,
                    stop=True,
                )
                col = colpool.tile([128, 1], f32)
                nc.scalar.copy(col, pw)
                nc.vector.tensor_scalar_mul(o_t[:, g, :], ps[:, 0:N], col)

            # store: per partition G*N = 1KB contiguous; Pool/SWDGE queue so
            # the wait on o_t doesn't block next chunk's loads on SP.
            nc.gpsimd.dma_start(out=out[:, c, :, :], in_=o_t)

    if split_waits:
        split_multi_waits(nc)
    return nc


_NC_CACHE = {}


def _get_nc():
    if "nc" not in _NC_CACHE:
        _NC_CACHE["nc"] = build()
    return _NC_CACHE["nc"]


def _pack_inputs(fi_v, d_av, c):
    fi = np.ascontiguousarray(fi_v[c * BPC : (c + 1) * BPC].transpose(1, 0, 2))
    d = np.ascontiguousarray(d_av[c * BPC : (c + 1) * BPC].transpose(1, 0, 2))
    return {"fi_t": fi, "d_t": d}


def _unpack_output(od, nchunk=BPC // E_CHUNK, g=E_CHUNK // 8):
    # od: [128, nchunk, G, N], partition p = 16*jj + s, e = c*E + g*8 + jj
    return (
        od.reshape(8, S, nchunk, g, N)
        .transpose(2, 3, 0, 1, 4)
        .reshape(BPC, S * N)
    )


def kernel(fi_v: np.ndarray, d_av: np.ndarray) -> np.ndarray:
    fi_v = np.asarray(fi_v, dtype=np.float32)
    d_av = np.asarray(d_av, dtype=np.float32)
    assert fi_v.shape == (B, V, N) and d_av.shape == (B, V, S)
    nc = _get_nc()
    in_maps = [_pack_inputs(fi_v, d_av, c) for c in range(NCORES)]
    res = run_bass_kernel_spmd(nc, in_maps, core_ids=list(range(NCORES)))
    return np.concatenate(
        [_unpack_output(np.asarray(res.results[c]["out"])) for c in range(NCORES)],
        axis=0,
    )
